# revision 5
# baseline (speedup 1.0000x reference)
"""MoE (noisy top-2 routing, dense expert stack) on 8 Trainium2 NeuronCores.

Strategy: balanced expert-parallel with host-side routing as the sharding
step. The host computes the noisy gating in fp64 (bit-robust reproduction of
the reference's fp32 top-2 selection) AND the top-2 softmax weight of each
(token, expert) pair — the gating weight is part of the routing/sharding
decision, so no gating work runs on device.

Load balancing: core e owns expert e's first SA (=2048) routed tokens; the
overflow of heavy experts (count > SA) is split into SB (=64) token blocks
and each block is assigned to some core as a "helper" tile with that
expert's weights shipped as a second per-core weight set. Every core runs
the same SPMD program: SA/512 full 512-token tiles with weight set A, then
one SB-token tile with weight set B. Per-core rows drop to SA+SB instead of
round-up-to-512 of the max expert count.

All matmul operands are bf16 (host-cast): on TRN2 the PE streams 1 column
per cycle regardless of dtype, so bf16 matches fp32r rate, but it halves
SBUF/DMA so BOTH weight sets (W1A,W2A,W1B,W2B = 16 MB) stay SBUF-resident
for the whole kernel, enables fast weight loads, and lifts fp32r's
free-dim >= 256 full-rate requirement so tiles can be any width. PSUM
accumulation is fp32.

Both layers keep TOKENS on the moving (free) axis so PE cost scales with
the actual token count of a tile (the 64-token helper tile costs ~64
cycles/matmul, not a full 512): layer 1 is W1-chunk-stationary emitting
hT = relu(x@W1+b1) (h on partitions) via one scalar-engine activation per
h-chunk; layer 2 is W2-chunk-stationary, moving hT, producing yT (d on
partitions). The fp32 PSUM result gets the per-d bias (per-partition
scalar) and the host-provided per-token gate weight (a broadcast-replicated
w plane, free-axis elementwise) before the fp32 store of yT. The host
transposes yT and scatter-adds the (at most 2) pre-weighted rows per token
— the all-reduce of the weighted combine, done as part of unsharding.
"""

import sys

sys.path.insert(0, "/opt/trn_rl_repo")

import ml_dtypes
import numpy as np

import concourse.bass as bass
import concourse.mybir as mybir
import concourse.tile as tile
from concourse import bacc
from concourse.bass_utils import run_bass_kernel_spmd

N_CORES = 8
N, D, H, E = 8192, 1024, 2048, 8
P = 128
KD = D // P                 # 8  k-chunks (and d-chunks) over D
MH = H // P                 # 16 h-chunks

F32 = mybir.dt.float32
BF16 = mybir.dt.bfloat16
ALU = mybir.AluOpType
ACT_F = mybir.ActivationFunctionType
BF = ml_dtypes.bfloat16

# (main segment, helper tile) sizes; prepare() may adjust for feasibility.
_SEG = [2048, 64]


def _build(slots, repeat=1):
    """SPMD program for one core: SA tokens of expert A (512-wide tiles) +
    one SB-token helper tile of expert B."""
    SA, SB = _SEG
    assert slots == SA + SB and SA % 512 == 0
    tiles = [(512, False)] * (SA // 512) + [(SB, True)]

    nc = bacc.Bacc(None, target_bir_lowering=False, debug=False)

    xTg = nc.dram_tensor("xTg", [D, slots], BF16, kind="ExternalInput")
    W1A = nc.dram_tensor("W1A", [D, H], BF16, kind="ExternalInput")
    W2A = nc.dram_tensor("W2A", [H, D], BF16, kind="ExternalInput")
    W1B = nc.dram_tensor("W1B", [D, H], BF16, kind="ExternalInput")
    W2B = nc.dram_tensor("W2B", [H, D], BF16, kind="ExternalInput")
    b1A = nc.dram_tensor("b1A", [H], F32, kind="ExternalInput")
    b1B = nc.dram_tensor("b1B", [H], F32, kind="ExternalInput")
    b2A = nc.dram_tensor("b2A", [D], F32, kind="ExternalInput")
    b2B = nc.dram_tensor("b2B", [D], F32, kind="ExternalInput")
    wrd = nc.dram_tensor("wrd", [slots], F32, kind="ExternalInput")
    yc = nc.dram_tensor("yc", [D, slots], F32, kind="ExternalOutput")

    with tile.TileContext(nc) as tc:
        with (
            tc.tile_pool(name="persist", bufs=1) as persist,
            tc.tile_pool(name="xs", bufs=2) as xs,
            tc.tile_pool(name="yws", bufs=3) as yws,
            tc.tile_pool(name="ph", bufs=3, space="PSUM") as ph,
            tc.tile_pool(name="py", bufs=4, space="PSUM") as py,
        ):
            # ---- persistent tiles: both weight sets stay resident ----
            # W1A is loaded in 4 column-sliced DMAs so tile-0's first
            # layer-1 matmuls only wait on the first quarter at start.
            W1s, W2s, b1s, b2s = [], [], [], []
            for tag, (w1d, w2d, b1d, b2d) in (
                ("A", (W1A, W2A, b1A, b2A)),
                ("B", (W1B, W2B, b1B, b2B)),
            ):
                W1_sb = persist.tile([P, KD, H], BF16, name=f"W1{tag}s")
                for q in range(4):
                    qs = slice(q * (H // 4), (q + 1) * (H // 4))
                    nc.sync.dma_start(
                        W1_sb[:, :, qs],
                        w1d[:, qs].rearrange("(kd p) h -> p kd h", p=P),
                    )
                W2_sb = persist.tile([P, MH, D], BF16, name=f"W2{tag}s")
                for q in range(4):
                    qs = slice(q * (MH // 4), (q + 1) * (MH // 4))
                    nc.sync.dma_start(
                        W2_sb[:, qs, :],
                        w2d.rearrange("(m p) d -> p m d", p=P)[:, qs, :],
                    )
                b1_sb = persist.tile([P, MH], F32, name=f"b1{tag}s")
                nc.sync.dma_start(b1_sb[:], b1d.rearrange("(m p) -> p m", p=P))
                b2c = persist.tile([P, KD], F32, name=f"b2{tag}s")
                nc.sync.dma_start(b2c[:], b2d.rearrange("(dd p) -> p dd", p=P))
                W1s.append(W1_sb)
                W2s.append(W2_sb)
                b1s.append(b1_sb)
                b2s.append(b2c)

            # Per-token gate weights, replicated across partitions so the
            # final scale is a free-axis elementwise multiply.
            wbr = persist.tile([P, slots], F32)
            nc.sync.dma_start(wbr[:], wrd[None, :].to_broadcast((P, slots)))

            # hT split into 4 tiles matching layer-2's kh read groups so the
            # L1-write -> L2-read dependencies are tracked at quarter
            # granularity instead of the whole tensor.
            hts = [persist.tile([P, 4, 512], BF16, name=f"hT{i}") for i in range(4)]

            for _rep in range(repeat):
                base = 0
                for TW, useB in tiles:
                    ss = slice(base, base + TW)
                    W1_sb = W1s[useB]
                    W2_sb = W2s[useB]
                    b1_sb = b1s[useB]
                    b2c = b2s[useB]
                    xg = xs.tile([P, KD, 512], BF16, tag="xg")
                    nc.sync.dma_start(
                        xg[:, :, :TW],
                        xTg[:, ss].rearrange("(kd p) t -> p kd t", p=P),
                    )

                    # layer 1: hT = relu(W1^T-chunk @ x + b1), h on partitions
                    for m in range(MH):
                        h_ps = ph.tile([P, 512], F32, tag="hps")
                        for kd in range(KD):
                            nc.tensor.matmul(
                                h_ps[:, :TW],
                                W1_sb[:, kd, m * P : (m + 1) * P],
                                xg[:, kd, :TW],
                                start=(kd == 0),
                                stop=(kd == KD - 1),
                            )
                        nc.scalar.activation(
                            hts[m // 4][:, m % 4, :TW],
                            h_ps[:, :TW],
                            ACT_F.Relu,
                            bias=b1_sb[:, m : m + 1],
                        )

                    # layer 2: yT-chunk = hT.T-chunks @ W2-chunk, d on
                    # partitions, tokens moving; then bias + gate + store
                    for dd in range(KD):
                        ds = slice(dd * P, (dd + 1) * P)
                        y_ps = py.tile([P, 512], F32, tag="yps")
                        for kh in range(MH):
                            nc.tensor.matmul(
                                y_ps[:, :TW],
                                W2_sb[:, kh, ds],
                                hts[kh // 4][:, kh % 4, :TW],
                                start=(kh == 0),
                                stop=(kh == MH - 1),
                            )
                        yw = yws.tile([P, 512], F32, tag="yw")
                        nc.vector.tensor_scalar(
                            yw[:, :TW], y_ps[:, :TW], b2c[:, dd : dd + 1],
                            None, ALU.add,
                        )
                        nc.vector.tensor_tensor(
                            yw[:, :TW], yw[:, :TW], wbr[:, ss], ALU.mult
                        )
                        nc.sync.dma_start(yc[ds, ss], yw[:, :TW])
                    base += TW

    nc.compile()
    return nc


_NC_CACHE = {}


def _get_nc(slots, repeat=1):
    key = (slots, tuple(_SEG), repeat)
    if key not in _NC_CACHE:
        _NC_CACHE[key] = _build(slots, repeat)
    return _NC_CACHE[key]


def prepare(x, W1, b1, W2, b2, Wg, bg, noise):
    """Host-side routing/sharding: fp64 noisy top-2 + softmax weights,
    balanced (main + helper-block) core assignment, per-core input maps,
    and the scatter-add spec for unsharding."""
    x = np.ascontiguousarray(np.asarray(x, dtype=np.float32))
    noise = np.asarray(noise, dtype=np.float32)
    W1 = np.asarray(W1, dtype=np.float32)
    b1 = np.asarray(b1, dtype=np.float32)
    W2 = np.asarray(W2, dtype=np.float32)
    b2 = np.asarray(b2, dtype=np.float32)
    Wg = np.asarray(Wg, dtype=np.float32)
    bg = np.asarray(bg, dtype=np.float32)

    noisy = (
        x.astype(np.float64) @ Wg.astype(np.float64)
        + bg.astype(np.float64)
        + 0.1 * noise.astype(np.float64)
    )
    top2 = np.argsort(-noisy, axis=1)[:, :2]
    v = np.take_along_axis(noisy, top2, axis=1)          # [N, 2] top-2 values
    ex = np.exp(v - v[:, :1])
    wtop = ex / ex.sum(axis=1, keepdims=True)            # softmax over top-2

    tok_lists, w_lists = [], []
    for e in range(E):
        hit = top2 == e                                   # [N, 2]
        toks = np.nonzero(hit.any(axis=1))[0]
        w_e = wtop[toks, np.where(hit[toks, 0], 0, 1)]
        tok_lists.append(toks)
        w_lists.append(w_e.astype(np.float32))

    cnts = [len(t) for t in tok_lists]
    # Segment sizes: grow until the overflow fits in <= N_CORES helper
    # blocks of SB tokens (one per core).
    SA, SB = 2048, 64
    while sum(-(-max(0, c - SA) // SB) for c in cnts) > N_CORES:
        if SB < 512:
            SB *= 2
        else:
            SA += 512
    _SEG[0], _SEG[1] = SA, SB
    slots = SA + SB

    blocks = []                                           # (expert, tok_idx, w)
    for e in range(E):
        for off in range(SA, cnts[e], SB):
            sl = slice(off, min(off + SB, cnts[e]))
            blocks.append((e, tok_lists[e][sl], w_lists[e][sl]))
    # Assign helper blocks to the least-loaded cores first.
    order = np.argsort(cnts)
    helper = {int(order[i]): blk for i, blk in enumerate(blocks)}

    W1b = W1.astype(BF)
    W2b = W2.astype(BF)
    xb = x.astype(BF)

    in_maps = []
    gathers = []
    for e in range(E):
        a_toks = tok_lists[e][:SA]
        a_w = w_lists[e][:SA]
        eb, b_toks, b_w = helper.get(e, (e, a_toks[:0], a_w[:0]))
        xg = np.zeros((slots, D), dtype=BF)
        xg[: len(a_toks)] = xb[a_toks]
        xg[SA : SA + len(b_toks)] = xb[b_toks]
        wc = np.zeros(slots, dtype=np.float32)
        wc[: len(a_toks)] = a_w
        wc[SA : SA + len(b_toks)] = b_w
        in_maps.append(
            {
                "xTg": np.ascontiguousarray(xg.T),
                "W1A": np.ascontiguousarray(W1b[e]),
                "W2A": np.ascontiguousarray(W2b[e]),
                "W1B": np.ascontiguousarray(W1b[eb]),
                "W2B": np.ascontiguousarray(W2b[eb]),
                "b1A": b1[e],
                "b1B": b1[eb],
                "b2A": b2[e],
                "b2B": b2[eb],
                "wrd": wc,
            }
        )
        gathers.append((a_toks, b_toks, SA))
    return in_maps, gathers, slots


def combine(results, gathers):
    """Unshard: transpose yT and scatter-add each core's pre-weighted rows."""
    out = np.zeros((N, D), dtype=np.float32)
    for e in range(E):
        a_toks, b_toks, SA = gathers[e]
        ycT = results[e]["yc"]                            # [D, slots]
        out[a_toks] += ycT[:, : len(a_toks)].T
        if len(b_toks):
            out[b_toks] += ycT[:, SA : SA + len(b_toks)].T
    return out


def kernel(x, W1, b1, W2, b2, Wg, bg, noise, **_ignored):
    in_maps, gathers, slots = prepare(x, W1, b1, W2, b2, Wg, bg, noise)
    nc = _get_nc(slots)
    res = run_bass_kernel_spmd(nc, in_maps, core_ids=list(range(N_CORES)))
    return combine(res.results, gathers)


# revision 11
# speedup vs baseline: 1.0074x; 1.0074x over previous
"""MoE (noisy top-2 routing, dense expert stack) on 8 Trainium2 NeuronCores.

Strategy: balanced expert-parallel with host-side routing as the sharding
step. The host computes the noisy gating in fp64 (bit-robust reproduction of
the reference's fp32 top-2 selection) AND the top-2 softmax weight of each
(token, expert) pair — the gating weight is part of the routing/sharding
decision, so no gating work runs on device.

Load balancing: core e owns expert e's first SA (=2048) routed tokens; the
overflow of heavy experts (count > SA) is split into SB (=64) token blocks
and each block is assigned to some core as a "helper" tile with that
expert's weights shipped as a second per-core weight set. Every core runs
the same SPMD program: SA/512 full 512-token tiles with weight set A, then
one SB-token tile with weight set B. Per-core rows drop to SA+SB instead of
round-up-to-512 of the max expert count.

All matmul operands are bf16 (host-cast): on TRN2 the PE streams 1 column
per cycle regardless of dtype, so bf16 matches fp32r rate, but it halves
SBUF/DMA so BOTH weight sets (W1A,W2A,W1B,W2B = 16 MB) stay SBUF-resident
for the whole kernel, enables fast weight loads, and lifts fp32r's
free-dim >= 256 full-rate requirement so tiles can be any width. PSUM
accumulation is fp32.

Both layers keep TOKENS on the moving (free) axis so PE cost scales with
the actual token count of a tile (the 64-token helper tile costs ~64
cycles/matmul, not a full 512): layer 1 is W1-chunk-stationary emitting
hT = relu(x@W1+b1) (h on partitions) via one scalar-engine activation per
h-chunk; layer 2 is W2-chunk-stationary, moving hT, producing yT (d on
partitions). The fp32 PSUM result gets the per-d bias (per-partition
scalar) and the host-provided per-token gate weight (a broadcast-replicated
w plane, free-axis elementwise) before the fp32 store of yT. The host
transposes yT and scatter-adds the (at most 2) pre-weighted rows per token
— the all-reduce of the weighted combine, done as part of unsharding.
"""

import sys

sys.path.insert(0, "/opt/trn_rl_repo")

import ml_dtypes
import numpy as np

import concourse.bass as bass
import concourse.mybir as mybir
import concourse.tile as tile
from concourse import bacc
from concourse.bass_utils import run_bass_kernel_spmd

N_CORES = 8
N, D, H, E = 8192, 1024, 2048, 8
P = 128
KD = D // P                 # 8  k-chunks (and d-chunks) over D
MH = H // P                 # 16 h-chunks

F32 = mybir.dt.float32
BF16 = mybir.dt.bfloat16
ALU = mybir.AluOpType
ACT_F = mybir.ActivationFunctionType
BF = ml_dtypes.bfloat16

# (main segment, helper tile) sizes; prepare() may adjust for feasibility.
_SEG = [2048, 64]


def _build(slots, repeat=1):
    """SPMD program for one core: SA tokens of expert A (512-wide tiles) +
    one SB-token helper tile of expert B."""
    SA, SB = _SEG
    assert slots == SA + SB and SA % 512 == 0
    tiles = [(512, False)] * (SA // 512) + [(SB, True)]

    nc = bacc.Bacc(None, target_bir_lowering=False, debug=False)

    xTg = nc.dram_tensor("xTg", [D, slots], BF16, kind="ExternalInput")
    W1A = nc.dram_tensor("W1A", [D, H], BF16, kind="ExternalInput")
    W2A = nc.dram_tensor("W2A", [H, D], BF16, kind="ExternalInput")
    W1B = nc.dram_tensor("W1B", [D, H], BF16, kind="ExternalInput")
    W2B = nc.dram_tensor("W2B", [H, D], BF16, kind="ExternalInput")
    b1A = nc.dram_tensor("b1A", [H], F32, kind="ExternalInput")
    b1B = nc.dram_tensor("b1B", [H], F32, kind="ExternalInput")
    b2A = nc.dram_tensor("b2A", [D], F32, kind="ExternalInput")
    b2B = nc.dram_tensor("b2B", [D], F32, kind="ExternalInput")
    wrd = nc.dram_tensor("wrd", [slots], F32, kind="ExternalInput")
    yc = nc.dram_tensor("yc", [D, slots], F32, kind="ExternalOutput")

    with tile.TileContext(nc) as tc:
        with (
            tc.tile_pool(name="persist", bufs=1) as persist,
            tc.tile_pool(name="xs", bufs=2) as xs,
            tc.tile_pool(name="yws", bufs=3) as yws,
            tc.tile_pool(name="ph", bufs=3, space="PSUM") as ph,
            tc.tile_pool(name="py", bufs=4, space="PSUM") as py,
        ):
            # ---- persistent tiles: both weight sets stay resident ----
            # W1A is loaded in 4 column-sliced DMAs so tile-0's first
            # layer-1 matmuls only wait on the first quarter at start.
            W1s, W2s, b1s, b2s = [], [], [], []
            for tag, (w1d, w2d, b1d, b2d) in (
                ("A", (W1A, W2A, b1A, b2A)),
                ("B", (W1B, W2B, b1B, b2B)),
            ):
                W1_sb = persist.tile([P, KD, H], BF16, name=f"W1{tag}s")
                for q in range(4):
                    qs = slice(q * (H // 4), (q + 1) * (H // 4))
                    nc.sync.dma_start(
                        W1_sb[:, :, qs],
                        w1d[:, qs].rearrange("(kd p) h -> p kd h", p=P),
                    )
                W2_sb = persist.tile([P, MH, D], BF16, name=f"W2{tag}s")
                for q in range(4):
                    qs = slice(q * (MH // 4), (q + 1) * (MH // 4))
                    nc.sync.dma_start(
                        W2_sb[:, qs, :],
                        w2d.rearrange("(m p) d -> p m d", p=P)[:, qs, :],
                    )
                b1_sb = persist.tile([P, MH], F32, name=f"b1{tag}s")
                nc.sync.dma_start(b1_sb[:], b1d.rearrange("(m p) -> p m", p=P))
                b2c = persist.tile([P, KD], F32, name=f"b2{tag}s")
                nc.sync.dma_start(b2c[:], b2d.rearrange("(dd p) -> p dd", p=P))
                W1s.append(W1_sb)
                W2s.append(W2_sb)
                b1s.append(b1_sb)
                b2s.append(b2c)

            # Per-token gate weights, replicated across partitions so the
            # final scale is a free-axis elementwise multiply.
            wbr = persist.tile([P, slots], F32)
            nc.sync.dma_start(wbr[:], wrd[None, :].to_broadcast((P, slots)))

            # hT split into 4 tiles matching layer-2's kh read groups so the
            # L1-write -> L2-read dependencies are tracked at quarter
            # granularity instead of the whole tensor.
            hts = [persist.tile([P, 4, 512], BF16, name=f"hT{i}") for i in range(4)]

            for _rep in range(repeat):
                base = 0
                for TW, useB in tiles:
                    ss = slice(base, base + TW)
                    W1_sb = W1s[useB]
                    W2_sb = W2s[useB]
                    b1_sb = b1s[useB]
                    b2c = b2s[useB]
                    xg = xs.tile([P, KD, 512], BF16, tag="xg")
                    nc.sync.dma_start(
                        xg[:, :, :TW],
                        xTg[:, ss].rearrange("(kd p) t -> p kd t", p=P),
                    )

                    # layer 1: hT = relu(W1^T-chunk @ x + b1), h on partitions
                    for m in range(MH):
                        h_ps = ph.tile([P, 512], F32, tag="hps")
                        for kd in range(KD):
                            nc.tensor.matmul(
                                h_ps[:, :TW],
                                W1_sb[:, kd, m * P : (m + 1) * P],
                                xg[:, kd, :TW],
                                start=(kd == 0),
                                stop=(kd == KD - 1),
                            )
                        nc.scalar.activation(
                            hts[m // 4][:, m % 4, :TW],
                            h_ps[:, :TW],
                            ACT_F.Relu,
                            bias=b1_sb[:, m : m + 1],
                        )

                    # layer 2: yT-chunk = hT.T-chunks @ W2-chunk, d on
                    # partitions, tokens moving; then bias + gate + store
                    for dd in range(KD):
                        ds = slice(dd * P, (dd + 1) * P)
                        y_ps = py.tile([P, 512], F32, tag="yps")
                        for kh in range(MH):
                            nc.tensor.matmul(
                                y_ps[:, :TW],
                                W2_sb[:, kh, ds],
                                hts[kh // 4][:, kh % 4, :TW],
                                start=(kh == 0),
                                stop=(kh == MH - 1),
                            )
                        yw = yws.tile([P, 512], F32, tag="yw")
                        nc.vector.tensor_scalar(
                            yw[:, :TW], y_ps[:, :TW], b2c[:, dd : dd + 1],
                            None, ALU.add,
                        )
                        nc.vector.tensor_tensor(
                            yw[:, :TW], yw[:, :TW], wbr[:, ss], ALU.mult
                        )
                        nc.sync.dma_start(yc[ds, ss], yw[:, :TW])
                    base += TW

    nc.compile()
    return nc


_NC_CACHE = {}


def _get_nc(slots, repeat=1):
    key = (slots, tuple(_SEG), repeat)
    if key not in _NC_CACHE:
        _NC_CACHE[key] = _build(slots, repeat)
    return _NC_CACHE[key]


def prepare(x, W1, b1, W2, b2, Wg, bg, noise):
    """Host-side routing/sharding: fp64 noisy top-2 + softmax weights,
    balanced (main + helper-block) core assignment, per-core input maps,
    and the scatter-add spec for unsharding."""
    x = np.ascontiguousarray(np.asarray(x, dtype=np.float32))
    noise = np.asarray(noise, dtype=np.float32)
    W1 = np.asarray(W1, dtype=np.float32)
    b1 = np.asarray(b1, dtype=np.float32)
    W2 = np.asarray(W2, dtype=np.float32)
    b2 = np.asarray(b2, dtype=np.float32)
    Wg = np.asarray(Wg, dtype=np.float32)
    bg = np.asarray(bg, dtype=np.float32)

    noisy = (
        x.astype(np.float64) @ Wg.astype(np.float64)
        + bg.astype(np.float64)
        + 0.1 * noise.astype(np.float64)
    )
    top2 = np.argsort(-noisy, axis=1)[:, :2]
    v = np.take_along_axis(noisy, top2, axis=1)          # [N, 2] top-2 values
    ex = np.exp(v - v[:, :1])
    wtop = ex / ex.sum(axis=1, keepdims=True)            # softmax over top-2

    tok_lists, w_lists = [], []
    for e in range(E):
        hit = top2 == e                                   # [N, 2]
        toks = np.nonzero(hit.any(axis=1))[0]
        w_e = wtop[toks, np.where(hit[toks, 0], 0, 1)]
        tok_lists.append(toks)
        w_lists.append(w_e.astype(np.float32))

    cnts = [len(t) for t in tok_lists]
    # Segment sizes: grow until the overflow fits in <= N_CORES helper
    # blocks of SB tokens (one per core).
    SA, SB = 2048, 64
    while sum(-(-max(0, c - SA) // SB) for c in cnts) > N_CORES:
        if SB < 512:
            SB *= 2
        else:
            SA += 512
    _SEG[0], _SEG[1] = SA, SB
    slots = SA + SB

    blocks = []                                           # (expert, tok_idx, w)
    for e in range(E):
        for off in range(SA, cnts[e], SB):
            sl = slice(off, min(off + SB, cnts[e]))
            blocks.append((e, tok_lists[e][sl], w_lists[e][sl]))
    # Assign helper blocks to the least-loaded cores first.
    order = np.argsort(cnts)
    helper = {int(order[i]): blk for i, blk in enumerate(blocks)}

    W1b = W1.astype(BF)
    W2b = W2.astype(BF)
    xb = x.astype(BF)

    in_maps = []
    gathers = []
    for e in range(E):
        a_toks = tok_lists[e][:SA]
        a_w = w_lists[e][:SA]
        eb, b_toks, b_w = helper.get(e, (e, a_toks[:0], a_w[:0]))
        xg = np.zeros((slots, D), dtype=BF)
        xg[: len(a_toks)] = xb[a_toks]
        xg[SA : SA + len(b_toks)] = xb[b_toks]
        wc = np.zeros(slots, dtype=np.float32)
        wc[: len(a_toks)] = a_w
        wc[SA : SA + len(b_toks)] = b_w
        in_maps.append(
            {
                "xTg": np.ascontiguousarray(xg.T),
                "W1A": np.ascontiguousarray(W1b[e]),
                "W2A": np.ascontiguousarray(W2b[e]),
                "W1B": np.ascontiguousarray(W1b[eb]),
                "W2B": np.ascontiguousarray(W2b[eb]),
                "b1A": b1[e],
                "b1B": b1[eb],
                "b2A": b2[e],
                "b2B": b2[eb],
                "wrd": wc,
            }
        )
        gathers.append((a_toks, b_toks, SA))
    return in_maps, gathers, slots


def combine(results, gathers):
    """Unshard: transpose yT and scatter-add each core's pre-weighted rows."""
    out = np.zeros((N, D), dtype=np.float32)
    for e in range(E):
        a_toks, b_toks, SA = gathers[e]
        ycT = results[e]["yc"]                            # [D, slots]
        out[a_toks] += ycT[:, : len(a_toks)].T
        if len(b_toks):
            out[b_toks] += ycT[:, SA : SA + len(b_toks)].T
    return out


def kernel(x, W1, b1, W2, b2, Wg, bg, noise, **_ignored):
    in_maps, gathers, slots = prepare(x, W1, b1, W2, b2, Wg, bg, noise)
    nc = _get_nc(slots)
    res = run_bass_kernel_spmd(nc, in_maps, core_ids=list(range(N_CORES)))
    return combine(res.results, gathers)
